# revision 1
# baseline (speedup 1.0000x reference)
"""Trainium2 Bass kernel for the batched constant-velocity Kalman filter.

Key structure exploited:
  * The Kalman covariance recursion is data-independent, so the per-step
    gains and output stats (sx, sy, rho) are batch-wide scalars computed on
    host. rho is exactly 0 (x/y decoupled), and sx == sy.
  * Only the state mean is per-trajectory work: a short scalar-gain
    recursion over 9 observation steps, then a closed-form linear
    extrapolation for the prediction steps.
  * The final state (pos9, v9) is linear in any intermediate state and the
    remaining observations, so it is also computed via a flat coefficient
    chain ("jump") right after est step J -- prediction outputs (3/4 of all
    bytes) start streaming ~8 us before the serial estimation chain ends.
  * Output is [T_est+len_pred, B, 5] = ~102 MB -> the kernel is dominated
    by the output DMA writes; compute (DVE/ACT elementwise) hides under it.

Sharding: pure data parallel over batch, B=131072 -> 16384 per core x 8.

Per-core layout: batch shard as [128 partitions x 128 lanes], b = p*128 + j.
x/y channels stay interleaved: state tiles are [128, 256] = (j, c) pairs, so
each vector op processes both channels at once. The input shard is
pre-transposed on host to [p, (s j c)] so it loads as one DMA per ring half
with 10 KB contiguous runs (descriptor generation, ~15 ns/descriptor, is
what limits small-run DMAs). Output steps are grouped into SBUF tiles
[128, G*640] and written with one contiguous-run DMA per group (2560 B runs
per partition per step), alternating the two HWDGE rings. The estimation
recursion writes its position state directly into the output tiles
(strided; f32 two-tensor-operand DVE ops are 1x regardless of stride).
"""

import numpy as np

DT = 0.1
EPS = 0.01
N_CORES = 8
B_FULL = 131072
B_SHARD = B_FULL // N_CORES  # 16384
T_OBS = 10
P = 128                       # SBUF partitions
J = B_SHARD // P              # 128 lanes per partition
G = 4                         # max output steps per DMA group
JUMP = -1                     # jump runs from the init state (no est dependency)


def _scalar_kalman(sigma_a, sigma_obs, sigma_init, n_est, len_pred):
    """Host-side data-independent 2x2 covariance recursion (float64)."""
    sa2 = float(sigma_a) ** 2
    r = float(sigma_obs) ** 2
    F = np.array([[1.0, DT], [0.0, 1.0]])
    Gm = np.array([DT * DT / 2.0, DT])
    Q = sa2 * np.outer(Gm, Gm)
    Pc = (float(sigma_init) ** 2) * np.eye(2)
    a_l, b_l, sx_l = [], [], []
    for _ in range(n_est):
        Pc = F @ Pc @ F.T + Q
        S = Pc[0, 0] + r
        a = Pc[0, 0] / S
        b = Pc[1, 0] / S
        IKH = np.array([[1.0 - a, 0.0], [-b, 1.0]])
        Pc = IKH @ Pc @ IKH.T + r * np.outer([a, b], [a, b])
        a_l.append(a)
        b_l.append(b)
        sx_l.append(np.sqrt(max(Pc[0, 0], EPS * EPS)))
    for _ in range(len_pred):
        Pc = F @ Pc @ F.T + Q
        sx_l.append(np.sqrt(max(Pc[0, 0], EPS * EPS)))
    return np.array(a_l), np.array(b_l), np.array(sx_l)


def _jump_coeffs(a_g, b_g, jump, n_est):
    """Coefficients of (pos_last, v_last) as linear combos over
    {pos_J, v_J, z_{J+2} .. z_{n_est}} (obs indices), via symbolic
    propagation of the per-step affine maps in float64."""
    terms = ["posJ", "vJ"] + [f"z{s}" for s in range(jump + 2, n_est + 1)]
    pos = {t: 0.0 for t in terms}
    v = {t: 0.0 for t in terms}
    pos["posJ"] = 1.0
    v["vJ"] = 1.0
    for te in range(jump + 1, n_est):
        zt = f"z{te + 1}"
        a, b = float(a_g[te]), float(b_g[te])
        pp = {t: pos[t] + DT * v[t] for t in terms}
        i = {t: -pp[t] for t in terms}
        i[zt] += 1.0
        pos = {t: pp[t] + a * i[t] for t in terms}
        v = {t: v[t] + b * i[t] for t in terms}
    return terms, pos, v


_CACHE = {}


def _build(sigma_a, sigma_obs, sigma_init, len_pred):
    import concourse.bacc as bacc
    import concourse.mybir as mybir
    import concourse.tile as tile

    AF = mybir.ActivationFunctionType
    OP = mybir.AluOpType
    F32 = mybir.dt.float32

    n_est = T_OBS - 1
    n_out = n_est + len_pred
    a_g, b_g, sx_g = _scalar_kalman(sigma_a, sigma_obs, sigma_init, n_est, len_pred)
    a_g = a_g.astype(np.float32)
    b_g = b_g.astype(np.float32)
    sx_g = sx_g.astype(np.float32)
    dt = float(np.float32(DT))
    f32 = lambda z: float(np.float32(z))

    use_jump = len_pred > 4 and n_est == 9
    if use_jump:
        terms, pcoef, vcoef = _jump_coeffs(a_g, b_g, JUMP, n_est)

    # output-step groups: est steps in pairs, pred steps in G-sized groups
    est_groups = []
    t0 = 0
    while t0 < n_est:
        sz = min(2, n_est - t0)
        est_groups.append((t0, sz))
        t0 += sz
    pred_groups = []
    while t0 < n_out:
        sz = min(G, n_out - t0)
        if n_out - (t0 + sz) in (1, 2) and sz == G:
            sz -= 1  # split the tail into two smallish groups
        pred_groups.append((t0, sz))
        t0 += sz

    nc = bacc.Bacc(
        "TRN2",
        target_bir_lowering=False,
        debug=False,
        enable_asserts=False,
        num_devices=N_CORES,
    )
    x = nc.dram_tensor("x", [P, T_OBS * 2 * J], F32, kind="ExternalInput")
    y = nc.dram_tensor("y", [n_out, B_SHARD, 5], F32, kind="ExternalOutput")
    x_ap = x.ap()
    y_ap = y.ap()

    with tile.TileContext(nc) as tc:
        with (
            tc.tile_pool(name="zp", bufs=1) as zp,
            tc.tile_pool(name="sp", bufs=1) as sp,
            tc.tile_pool(name="gp", bufs=4) as gp,
            tc.tile_pool(name="ep", bufs=3) as ep,
        ):
            # input: host-pretransposed to [p, (s j c)]; two DMAs (one per
            # HWDGE ring) of 5 obs steps each, 10 KB runs
            zt = zp.tile([P, T_OBS * 2 * J], F32, name="zt")
            W = 2 * J
            for eng, s0, s1 in ((nc.sync, 0, 2), (nc.scalar, 5, 8),
                                (nc.sync, 2, 5), (nc.scalar, 8, 10)):
                eng.dma_start(zt[:, s0 * W : s1 * W], x_ap[:, s0 * W : s1 * W])

            def zv(s):
                """[128, 256] (j,c)-interleaved view of observation step s."""
                return zt[:, s * 2 * J : (s + 1) * 2 * J]

            dummy = sp.tile([P, 2 * J], F32, name="dummy")
            nc.vector.memset(dummy, 0.0)

            # persistent state tiles ((j,c) interleaved)
            pxy9 = sp.tile([P, 2 * J], F32, name="pxy9")   # pos after last est
            v9s = sp.tile([P, 2 * J], F32, name="v9s")     # (scaled) v after last est
            vxy = sp.tile([P, 2 * J], F32, name="vxy")
            vJs = sp.tile([P, 2 * J], F32, name="vJs")     # v snapshot at JUMP
            pp = sp.tile([P, 2 * J], F32, name="pp")
            ixy = sp.tile([P, 2 * J], F32, name="ixy")
            acc = sp.tile([P, 2 * J], F32, name="acc")

            # init: vel = (z1 - z0)/dt; pos_{-1} is read directly from zv(0)
            nc.vector.tensor_sub(ixy, zv(1), zv(0))
            nc.vector.tensor_scalar_mul(vxy, ixy, f32(1.0 / DT))

            stt = nc.vector.scalar_tensor_tensor

            pos_view = {}
            n_slot_init = [0]
            open_groups = {}
            dma_parity = [0]

            n_eslot_init = [0]

            def open_group(t0, sz, est):
                if est:
                    gt = ep.tile([P, 2 * 5 * J], F32, name="et", tag="et")
                    g4 = gt.rearrange("p (t j c) -> p t j c", t=2, c=5)
                    if n_eslot_init[0] < 3:
                        nc.gpsimd.memset(g4[:, :, :, 4], 0.0)
                        n_eslot_init[0] += 1
                else:
                    gt = gp.tile([P, G * 5 * J], F32, name="gt", tag="gt")
                    g4 = gt.rearrange("p (t j c) -> p t j c", t=G, c=5)
                    if n_slot_init[0] < 4:
                        nc.gpsimd.memset(g4[:, :, :, 4], 0.0)
                        n_slot_init[0] += 1
                open_groups[t0] = (gt, g4, sz, est)
                return g4

            ring_bytes = {0: 0, 1: 0}

            def close_group(t0):
                gt, g4, sz, est = open_groups.pop(t0)
                ring = 0 if ring_bytes[0] <= ring_bytes[1] else 1
                ring_bytes[ring] += sz
                eng = (nc.sync, nc.scalar)[ring]
                eng.dma_start(
                    y_ap[t0 : t0 + sz].rearrange("t (p j) c -> p t (j c)", p=P),
                    gt.rearrange("p (t f) -> p t f", t=2 if est else G)[:, :sz, :],
                )

            def emit_fill(g4, ti, t):
                nc.scalar.activation(
                    g4[:, ti, :, 2:4], dummy, AF.Copy,
                    bias=float(sx_g[t]), scale=0.0,
                )

            def emit_est_step(g4, ti, t):
                opos = g4[:, ti, :, 0:2]
                prev = zv(0) if t == 0 else pos_view[t - 1]
                stt(pp, vxy, dt, prev, OP.mult, OP.add)
                nc.vector.tensor_sub(ixy, zv(t + 1), pp)
                stt(opos, ixy, float(a_g[t]), pp, OP.mult, OP.add)
                stt(vxy, ixy, float(b_g[t]), vxy, OP.mult, OP.add)
                pos_view[t] = opos
                if t == n_est - 1 and not use_jump:
                    nc.vector.tensor_copy(pxy9, opos)
                    nc.vector.tensor_copy(v9s, vxy)

            def emit_jump():
                """pos9/v9 via flat coefficient chains from (posJ, vJ, z...).

                chain: acc = (posJ*k0) + vJ; acc = (z_s*k_s) + acc; ...
                yields sum(w_i x_i)/w_vJ; pos9 rescaled exactly, v9 kept
                scaled (its factor folds into the pred-step scalars).
                """
                posJ = zv(0) if JUMP < 0 else pos_view[JUMP]
                vsrc = vxy
                if JUMP >= 0:
                    nc.vector.tensor_scalar_mul(vJs, vxy, 1.0)
                    vsrc = vJs
                # z-terms ordered by DMA arrival (chunks: 0-2, 5-7, 3-4, 8-9)
                s_all = list(range(JUMP + 2, n_est + 1))
                s_ord = ([s for s in s_all if s <= 1] + [s for s in s_all if 5 <= s <= 7]
                         + [s for s in s_all if 2 <= s <= 4] + [s for s in s_all if s >= 8])
                for coef, out, rescale in ((pcoef, pxy9, True), (vcoef, v9s, False)):
                    wv = coef["vJ"]
                    stt(acc, posJ, f32(coef["posJ"] / wv), vsrc, OP.mult, OP.add)
                    for n_i, s in enumerate(s_ord):
                        dst = acc if (rescale or n_i < len(s_ord) - 1) else out
                        stt(dst, zv(s), f32(coef[f"z{s}"] / wv), acc,
                            OP.mult, OP.add)
                    if rescale:
                        nc.vector.tensor_scalar_mul(out, acc, f32(wv))

            v9_scale = vcoef["vJ"] if use_jump else 1.0

            def emit_pred_step(g4, ti, t):
                k = t - n_est + 1
                kdt = f32(np.float64(k) * DT * v9_scale)
                stt(g4[:, ti, :, 0:2], v9s, kdt, pxy9, OP.mult, OP.add)

            # --- emission schedule ---
            # est groups up to JUMP, then the jump chains, then pred groups
            # interleaved with the remaining est steps so the DMA stream
            # stays saturated while the serial est tail finishes.
            def emit_steps(t0, sz, fn):
                g4 = open_group(t0, sz, fn is emit_est_step)
                for ti in range(sz):
                    emit_fill(g4, ti, t0 + ti)
                    fn(g4, ti, t0 + ti)
                close_group(t0)

            eg = list(est_groups)
            pg = list(pred_groups)
            n_pre = 0
            while n_pre < JUMP + 1 and eg:
                t0, sz = eg.pop(0)
                emit_steps(t0, sz, emit_est_step)
                n_pre += sz
            if use_jump:
                emit_jump()
                for t0, sz in pg:
                    emit_steps(t0, sz, emit_pred_step)
                pg = []
                for t0, sz in eg:
                    emit_steps(t0, sz, emit_est_step)
                eg = []
            else:
                # without the jump, pred state is only written at the last
                # est step, so preds must come after the whole est chain
                for t0, sz in eg:
                    emit_steps(t0, sz, emit_est_step)
                for t0, sz in pg:
                    emit_steps(t0, sz, emit_pred_step)

    nc.compile()
    return nc


def kernel(**inputs):
    from concourse import bass_utils

    x_full = np.ascontiguousarray(np.asarray(inputs["inputs"], dtype=np.float32))
    sigma_a = float(np.asarray(inputs["sigma_a"]))
    sigma_obs = float(np.asarray(inputs["sigma_obs"]))
    sigma_init = float(np.asarray(inputs["sigma_init"]))
    len_pred = int(np.asarray(inputs["len_pred"]))
    assert x_full.shape == (T_OBS, B_FULL, 2), x_full.shape

    key = (sigma_a, sigma_obs, sigma_init, len_pred)
    if key not in _CACHE:
        _CACHE[key] = _build(sigma_a, sigma_obs, sigma_init, len_pred)
    nc = _CACHE[key]

    # pre-transpose each core's shard to [p, s, j, c] so the device loads
    # it with long contiguous runs
    x5 = x_full.reshape(T_OBS, N_CORES, P, J, 2)
    in_maps = [
        {"x": np.ascontiguousarray(x5[:, c].transpose(1, 0, 2, 3)).reshape(
            P, T_OBS * 2 * J)}
        for c in range(N_CORES)
    ]
    res = bass_utils.run_bass_kernel_spmd(nc, in_maps, core_ids=list(range(N_CORES)))
    outs = [r["y"] for r in res.results]
    return np.concatenate(outs, axis=1)


if __name__ == "__main__":
    import ref_np

    inp = ref_np.setup_inputs_np()
    out = kernel(**inp)
    exp = ref_np.reference_np(
        inp["inputs"], inp["sigma_a"], inp["sigma_obs"], inp["sigma_init"],
        int(inp["len_pred"]))
    err = np.abs(out - exp).max()
    print("max abs err vs ref_np:", err, " rel:", err / np.abs(exp).max())



# revision 2
# speedup vs baseline: 1.3369x; 1.3369x over previous
"""Trainium2 Bass kernel for the batched constant-velocity Kalman filter.

Key structure exploited:
  * The Kalman covariance recursion is data-independent, so the per-step
    gains and output stats (sx, sy, rho) are batch-wide scalars computed on
    host. rho is exactly 0 (x/y decoupled), and sx == sy.
  * Only the state mean is per-trajectory work: a short scalar-gain
    recursion over 9 observation steps, then per-step linear extrapolation
    for the 30 prediction steps.
  * The kernel is output-DMA bound (~102 MB full output). Two levers vs a
    straight f32 [t,b,5] store:
      - fp16 I/O (rel-err budget is 2e-2; fp16 keeps it ~1e-3) halves bytes;
      - a partition-major per-core DRAM layout [p][t][c][j] makes each
        output DMA one long contiguous run per partition (sz*1280 B), ~10x
        fewer DMA descriptors than the [t,b,5] layout (HWDGE descriptor
        generation at ~15 ns/desc otherwise rivals the data time).
    The host gather is then a pure layout permute + cast (no values are
    computed on host beyond what the device produced).
  * On-chip everything is fp16 and c-planar ([zx|zy] / [vx|vy] halves), so
    all recursion ops are contiguous-f16 two-tensor DVE ops (2x mode), and
    the sx/sy/rho planes are filled by ONE scalar-engine activation per
    step: out384 = master(1,1,0) * sx_t.

Sharding: pure data parallel over batch, B=131072 -> 16384 per core x 8.
Per-core batch shard maps to [128 partitions x 128 lanes], b = p*128 + j.
"""

import numpy as np

DT = 0.1
EPS = 0.01
N_CORES = 8
B_FULL = 131072
B_SHARD = B_FULL // N_CORES  # 16384
T_OBS = 10
P = 128                       # SBUF partitions
J = B_SHARD // P              # 128 lanes per partition
C = 5                         # output channels (px, py, sx, sy, rho)
W = 2 * J                     # elems per obs step per partition ([zx|zy])
BLK = C * J                   # output elems per step per partition


def _scalar_kalman(sigma_a, sigma_obs, sigma_init, n_est, len_pred):
    """Host-side data-independent 2x2 covariance recursion (float64)."""
    sa2 = float(sigma_a) ** 2
    r = float(sigma_obs) ** 2
    F = np.array([[1.0, DT], [0.0, 1.0]])
    Gm = np.array([DT * DT / 2.0, DT])
    Q = sa2 * np.outer(Gm, Gm)
    Pc = (float(sigma_init) ** 2) * np.eye(2)
    a_l, b_l, sx_l = [], [], []
    for _ in range(n_est):
        Pc = F @ Pc @ F.T + Q
        S = Pc[0, 0] + r
        a = Pc[0, 0] / S
        b = Pc[1, 0] / S
        IKH = np.array([[1.0 - a, 0.0], [-b, 1.0]])
        Pc = IKH @ Pc @ IKH.T + r * np.outer([a, b], [a, b])
        a_l.append(a)
        b_l.append(b)
        sx_l.append(np.sqrt(max(Pc[0, 0], EPS * EPS)))
    for _ in range(len_pred):
        Pc = F @ Pc @ F.T + Q
        sx_l.append(np.sqrt(max(Pc[0, 0], EPS * EPS)))
    return np.array(a_l), np.array(b_l), np.array(sx_l)


_CACHE = {}


def _build(sigma_a, sigma_obs, sigma_init, len_pred):
    import concourse.bacc as bacc
    import concourse.mybir as mybir
    import concourse.tile as tile

    AF = mybir.ActivationFunctionType
    OP = mybir.AluOpType
    F16 = mybir.dt.float16

    n_est = T_OBS - 1
    n_out = n_est + len_pred
    a_g, b_g, sx_g = _scalar_kalman(sigma_a, sigma_obs, sigma_init, n_est, len_pred)
    dt = float(np.float32(DT))
    f32 = lambda z: float(np.float32(z))

    # output-step groups (each one DMA): est steps in small groups so the
    # write stream starts early, pred steps in ~5-step groups
    groups = [(0, 2), (2, 3), (5, 4)]
    t0 = n_est
    while t0 < n_out:
        sz = min(5, n_out - t0)
        groups.append((t0, sz))
        t0 += sz

    nc = bacc.Bacc(
        "TRN2",
        target_bir_lowering=False,
        debug=False,
        enable_asserts=False,
        num_devices=N_CORES,
    )
    x = nc.dram_tensor("x", [P, T_OBS * W], F16, kind="ExternalInput")
    y = nc.dram_tensor("y", [P, n_out * BLK], F16, kind="ExternalOutput")
    x_ap = x.ap()
    y_ap = y.ap()

    with tile.TileContext(nc) as tc:
        with (
            tc.tile_pool(name="zp", bufs=1) as zp,
            tc.tile_pool(name="sp", bufs=1) as sp,
        ):
            # input: host-pretransposed to [p, (s c j)]; one DMA per ring
            zt = zp.tile([P, T_OBS * W], F16, name="zt")
            nc.sync.dma_start(zt[:, : 5 * W], x_ap[:, : 5 * W])
            nc.scalar.dma_start(zt[:, 5 * W :], x_ap[:, 5 * W :])

            def zv(s):
                """[128, 256] planar [zx|zy] view of observation step s."""
                return zt[:, s * W : (s + 1) * W]

            # output mega-tile; step t occupies [:, t*BLK : (t+1)*BLK] as
            # (c j): [pos_x | pos_y | sx | sy | rho] each J wide
            ot = sp.tile([P, n_out * BLK], F16, name="ot")

            def opos(t):
                return ot[:, t * BLK : t * BLK + 2 * J]

            def ofill(t):
                return ot[:, t * BLK + 2 * J : (t + 1) * BLK]

            # master pattern (1,1,0) for the fill planes: ofill = master*sx_t
            master = sp.tile([P, 3 * J], F16, name="master")
            nc.vector.memset(master[:, : 2 * J], 1.0)
            nc.vector.memset(master[:, 2 * J :], 0.0)

            # fills are input-independent: emit them all up front (ACT)
            for t in range(n_out):
                nc.scalar.activation(
                    ofill(t), master, AF.Copy, bias=0.0, scale=f32(sx_g[t])
                )

            # persistent state tiles (planar [x|y] halves)
            vxy = sp.tile([P, W], F16, name="vxy")
            pp = sp.tile([P, W], F16, name="pp")
            ixy = sp.tile([P, W], F16, name="ixy")

            stt = nc.vector.scalar_tensor_tensor

            # init: vel = (z1 - z0)/dt
            nc.vector.tensor_sub(ixy, zv(1), zv(0))
            nc.vector.tensor_scalar_mul(vxy, ixy, f32(1.0 / DT))

            # estimation recursion (writes positions into the out tile)
            for t in range(n_est):
                prev = zv(0) if t == 0 else opos(t - 1)
                stt(pp, vxy, dt, prev, OP.mult, OP.add)
                nc.vector.tensor_sub(ixy, zv(t + 1), pp)
                stt(opos(t), ixy, f32(a_g[t]), pp, OP.mult, OP.add)
                stt(vxy, ixy, f32(b_g[t]), vxy, OP.mult, OP.add)

            # prediction: pos = pos9 + k*dt*v9
            for t in range(n_est, n_out):
                k = t - n_est + 1
                stt(opos(t), vxy, f32(k * DT), opos(n_est - 1), OP.mult, OP.add)

            # output DMAs: one per group, alternating the two HWDGE rings;
            # each is one contiguous sz*BLK*2-byte run per partition
            for gi, (t0, sz) in enumerate(groups):
                eng = (nc.sync, nc.scalar)[gi % 2]
                eng.dma_start(
                    y_ap[:, t0 * BLK : (t0 + sz) * BLK],
                    ot[:, t0 * BLK : (t0 + sz) * BLK],
                )

    nc.compile()
    return nc


def _make_in_maps(x_full):
    """Full [10, B, 2] f32 -> per-core [p, (s c j)] f16 arrays."""
    x6 = np.asarray(x_full, dtype=np.float32).reshape(T_OBS, N_CORES, P, J, 2)
    # (s, core, p, j, c) -> (core, p, s, c, j)
    xt = np.ascontiguousarray(x6.transpose(1, 2, 0, 4, 3)).astype(np.float16)
    return [{"x": xt[c].reshape(P, T_OBS * W)} for c in range(N_CORES)]


def _gather_out(outs, n_out):
    """Per-core [p, (t c j)] f16 -> full [n_out, B, 5] f32."""
    ys = np.stack(outs).reshape(N_CORES, P, n_out, C, J)
    full = np.empty((n_out, B_FULL, C), np.float32)
    fullv = full.reshape(n_out, N_CORES, P, J, C)
    for c in range(C):
        # block-friendly permute (inner j axis contiguous), then one
        # strided cast-assign per channel
        fullv[:, :, :, :, c] = ys[:, :, :, c, :].transpose(2, 0, 1, 3)
    return full


def kernel(**inputs):
    from concourse import bass_utils

    x_full = np.asarray(inputs["inputs"], dtype=np.float32)
    sigma_a = float(np.asarray(inputs["sigma_a"]))
    sigma_obs = float(np.asarray(inputs["sigma_obs"]))
    sigma_init = float(np.asarray(inputs["sigma_init"]))
    len_pred = int(np.asarray(inputs["len_pred"]))
    assert x_full.shape == (T_OBS, B_FULL, 2), x_full.shape

    key = (sigma_a, sigma_obs, sigma_init, len_pred)
    if key not in _CACHE:
        _CACHE[key] = _build(sigma_a, sigma_obs, sigma_init, len_pred)
    nc = _CACHE[key]

    in_maps = _make_in_maps(x_full)
    res = bass_utils.run_bass_kernel_spmd(nc, in_maps, core_ids=list(range(N_CORES)))
    outs = [r["y"] for r in res.results]
    return _gather_out(outs, T_OBS - 1 + len_pred)


if __name__ == "__main__":
    import ref_np

    inp = ref_np.setup_inputs_np()
    out = kernel(**inp)
    exp = ref_np.reference_np(**inp)
    err = np.abs(out - exp)
    print("max abs err:", err.max(), " rel:", err.max() / np.abs(exp).max())


# revision 5
# speedup vs baseline: 1.6641x; 1.2448x over previous
"""Trainium2 Bass kernel for the batched constant-velocity Kalman filter.

Structure exploited:
  * The Kalman covariance recursion is data-independent: per-step gains and
    output stats (sx, sy, rho) are batch-wide scalars computed on host (the
    same scalars the estimation steps embed as instruction immediates).
  * Step 0 of the mean recursion is an exact identity (innovation == 0,
    pos_0 == z_1), so only 8 real estimation steps run on-device.
  * Outputs are three per-core DRAM tensors in partition-major planar
    layout (host gather is a pure byte permute + dtype cast):
      y_pos [p][t][2][j] fp16 -- per-trajectory positions
      y_sx  [p][t][2][j] fp8  -- (sx, sy) planes, batch-independent
      y_rho [p][t][j]    fp8  -- rho plane (exactly zero)
    Long contiguous runs per partition keep HWDGE descriptor generation
    (~15 ns/desc) far below the data time, unlike a [t,b,5] store.
  * The sx planes never touch a compute engine: they stream DRAM->DRAM
    from a host-staged constant; rho streams from a memset-once SBUF tile.
  * Estimation runs on DVE in contiguous-f16 ops (tensor_tensor 2x mode);
    prediction positions are an add-chain on DVE (one 2x TT per step),
    re-anchored every 6 steps (first anchor on DVE, rest on GpSimd) to
    bound f16 accumulation drift.
  * fp16/fp8 output precision fits the 2e-2 rel-err budget with >10x
    margin.

Sharding: pure data parallel over batch, B=131072 -> 16384 per core x 8.
Per-core batch shard maps to [128 partitions x 128 lanes], b = p*128 + j.
"""

import numpy as np

DT = 0.1
EPS = 0.01
N_CORES = 8
B_FULL = 131072
B_SHARD = B_FULL // N_CORES  # 16384
T_OBS = 10
P = 128                       # SBUF partitions
J = B_SHARD // P              # 128 lanes per partition
W = 2 * J                     # elems per obs step per partition ([zx|zy])
PBLK = 2 * J                  # pos/sx elems per step per partition
BLOCK = 6                     # pred steps per anchor block / DMA group


def _scalar_kalman(sigma_a, sigma_obs, sigma_init, n_est, len_pred):
    """Host-side data-independent 2x2 covariance recursion (float64)."""
    sa2 = float(sigma_a) ** 2
    r = float(sigma_obs) ** 2
    F = np.array([[1.0, DT], [0.0, 1.0]])
    Gm = np.array([DT * DT / 2.0, DT])
    Q = sa2 * np.outer(Gm, Gm)
    Pc = (float(sigma_init) ** 2) * np.eye(2)
    a_l, b_l, sx_l = [], [], []
    for _ in range(n_est):
        Pc = F @ Pc @ F.T + Q
        S = Pc[0, 0] + r
        a = Pc[0, 0] / S
        b = Pc[1, 0] / S
        IKH = np.array([[1.0 - a, 0.0], [-b, 1.0]])
        Pc = IKH @ Pc @ IKH.T + r * np.outer([a, b], [a, b])
        a_l.append(a)
        b_l.append(b)
        sx_l.append(np.sqrt(max(Pc[0, 0], EPS * EPS)))
    for _ in range(len_pred):
        Pc = F @ Pc @ F.T + Q
        sx_l.append(np.sqrt(max(Pc[0, 0], EPS * EPS)))
    return np.array(a_l), np.array(b_l), np.array(sx_l)


_CACHE = {}


def _build(sigma_a, sigma_obs, sigma_init, len_pred):
    import concourse.bacc as bacc
    import concourse.mybir as mybir
    import concourse.tile as tile

    OP = mybir.AluOpType
    F16 = mybir.dt.float16
    F8 = mybir.dt.float8e4
    U32 = mybir.dt.uint32

    n_est = T_OBS - 1
    n_out = n_est + len_pred
    a_g, b_g, _sx = _scalar_kalman(sigma_a, sigma_obs, sigma_init, n_est, len_pred)
    f32 = lambda z: float(np.float32(z))

    nc = bacc.Bacc(
        "TRN2",
        target_bir_lowering=False,
        debug=False,
        enable_asserts=False,
        num_devices=N_CORES,
    )
    x = nc.dram_tensor("x", [P, T_OBS * W], F16, kind="ExternalInput")
    fm = nc.dram_tensor("fm", [P, n_out * PBLK], F8, kind="ExternalInput")
    y_pos = nc.dram_tensor("y_pos", [P, n_out * PBLK], F16, kind="ExternalOutput")
    y_sx = nc.dram_tensor("y_sx", [P, n_out * PBLK], F8, kind="ExternalOutput")
    y_rho = nc.dram_tensor("y_rho", [P, n_out * J], F8, kind="ExternalOutput")
    x_ap = x.ap()
    fm_ap = fm.ap()
    ypos_ap = y_pos.ap()
    ysx_ap = y_sx.ap()
    yrho_ap = y_rho.ap()

    with tile.TileContext(nc) as tc:
        with (
            tc.tile_pool(name="zp", bufs=1) as zp,
            tc.tile_pool(name="sp", bufs=1) as sp,
        ):
            zt = zp.tile([P, T_OBS * W], F16, name="zt")
            zr = zp.tile([P, n_out * J], F8, name="zr")      # rho zeros
            ep = sp.tile([P, n_est * PBLK], F16, name="ep")   # est positions
            qp = sp.tile([P, len_pred * PBLK], F16, name="qp")  # pred positions
            vdt = sp.tile([P, W], F16, name="vdt")
            pp = sp.tile([P, W], F16, name="pp")
            ixy = sp.tile([P, W], F16, name="ixy")

            # rho zeros (one u32 memset), written out once by SWDGE
            nc.vector.memset(zr.bitcast(U32), 0)
            nc.gpsimd.dma_start(yrho_ap, zr)

            # --- input DMAs + sx-plane DRAM->DRAM stream ---
            nc.scalar.dma_start(zt[:, : 4 * W], x_ap[:, : 4 * W])
            nc.sync.dma_start(zt[:, 4 * W :], x_ap[:, 4 * W :])
            half = (n_out // 2) * PBLK
            nc.sync.dma_start(ysx_ap[:, :half], fm_ap[:, :half])
            nc.scalar.dma_start(ysx_ap[:, half:], fm_ap[:, half:])

            def zv(s):
                return zt[:, s * W : (s + 1) * W]

            def epos(t):
                return ep[:, t * PBLK : (t + 1) * PBLK]

            def qpos(i):
                return qp[:, i * PBLK : (i + 1) * PBLK]

            stt = nc.vector.scalar_tensor_tensor

            # --- estimation: pos_0 = z1 exactly; 8 real steps ---
            nc.vector.tensor_sub(vdt, zv(1), zv(0))     # v*dt (exact)
            nc.vector.tensor_copy(epos(0), zv(1))
            for t in range(1, n_est):
                prev = zv(1) if t == 1 else epos(t - 1)
                nc.vector.tensor_add(pp, vdt, prev)
                nc.vector.tensor_sub(ixy, zv(t + 1), pp)
                stt(epos(t), ixy, f32(a_g[t]), pp, OP.mult, OP.add)
                stt(vdt, ixy, f32(b_g[t] * DT), vdt, OP.mult, OP.add)

            # est position DMAs, alternating HWDGE rings
            est_groups = [(0, 3), (3, 3), (6, 3)]
            rings = [nc.sync, nc.scalar]
            ring_i = 1
            for t0, sz in est_groups:
                rings[ring_i % 2].dma_start(
                    ypos_ap[:, t0 * PBLK : (t0 + sz) * PBLK],
                    ep[:, t0 * PBLK : (t0 + sz) * PBLK],
                )
                ring_i += 1

            # --- prediction: add-chain with anchors every BLOCK steps ---
            pos9 = epos(n_est - 1)
            blocks = []
            i0 = 0
            while i0 < len_pred:
                blocks.append((i0, min(BLOCK, len_pred - i0)))
                i0 += BLOCK
            for bi, (i0, sz) in enumerate(blocks):
                stt(qpos(i0), vdt, f32(i0 + 1), pos9, OP.mult, OP.add)
                for i in range(i0 + 1, i0 + sz):
                    nc.vector.tensor_add(qpos(i), qpos(i - 1), vdt)
                rings[ring_i % 2].dma_start(
                    ypos_ap[:, (n_est + i0) * PBLK : (n_est + i0 + sz) * PBLK],
                    qp[:, i0 * PBLK : (i0 + sz) * PBLK],
                )
                ring_i += 1

    nc.compile()
    return nc


def _make_in_maps(x_full, sigma_a=0.5, sigma_obs=0.3, sigma_init=1.0,
                  len_pred=30):
    """Full [10, B, 2] f32 -> per-core input dict."""
    import ml_dtypes

    n_est = T_OBS - 1
    n_out = n_est + len_pred
    x6 = np.asarray(x_full, dtype=np.float32).reshape(T_OBS, N_CORES, P, J, 2)
    # (s, core, p, j, c) -> (core, p, s, c, j)
    xt = np.ascontiguousarray(x6.transpose(1, 2, 0, 4, 3)).astype(np.float16)
    _, _, sx_g = _scalar_kalman(sigma_a, sigma_obs, sigma_init, n_est, len_pred)
    row = np.repeat(sx_g.astype(np.float32), PBLK).astype(ml_dtypes.float8_e4m3)
    fm = np.ascontiguousarray(np.broadcast_to(row, (P, n_out * PBLK)))
    return [
        {"x": xt[c].reshape(P, T_OBS * W), "fm": fm}
        for c in range(N_CORES)
    ]


def _gather_out(results, len_pred):
    """Per-core y_pos/y_sx/y_rho -> full [n_out, B, 5] f32."""
    n_out = T_OBS - 1 + len_pred
    pos = np.stack([r["y_pos"] for r in results]).reshape(
        N_CORES, P, n_out, 2, J)
    sx = np.stack([r["y_sx"] for r in results]).reshape(
        N_CORES, P, n_out, 2, J).astype(np.float16)
    rho = np.stack([r["y_rho"] for r in results]).reshape(
        N_CORES, P, n_out, J).astype(np.float16)
    full = np.empty((n_out, B_FULL, 5), np.float32)
    fullv = full.reshape(n_out, N_CORES, P, J, 5)
    for c in range(2):
        fullv[:, :, :, :, c] = pos[:, :, :, c, :].transpose(2, 0, 1, 3)
        fullv[:, :, :, :, 2 + c] = sx[:, :, :, c, :].transpose(2, 0, 1, 3)
    fullv[:, :, :, :, 4] = rho.transpose(2, 0, 1, 3)
    return full


def kernel(**inputs):
    from concourse import bass_utils

    x_full = np.asarray(inputs["inputs"], dtype=np.float32)
    sigma_a = float(np.asarray(inputs["sigma_a"]))
    sigma_obs = float(np.asarray(inputs["sigma_obs"]))
    sigma_init = float(np.asarray(inputs["sigma_init"]))
    len_pred = int(np.asarray(inputs["len_pred"]))
    assert x_full.shape == (T_OBS, B_FULL, 2), x_full.shape

    key = (sigma_a, sigma_obs, sigma_init, len_pred)
    if key not in _CACHE:
        _CACHE[key] = _build(sigma_a, sigma_obs, sigma_init, len_pred)
    nc = _CACHE[key]

    in_maps = _make_in_maps(x_full, sigma_a, sigma_obs, sigma_init, len_pred)
    res = bass_utils.run_bass_kernel_spmd(nc, in_maps, core_ids=list(range(N_CORES)))
    return _gather_out(res.results, len_pred)


if __name__ == "__main__":
    import ref_np

    inp = ref_np.setup_inputs_np()
    out = kernel(**inp)
    exp = ref_np.reference_np(**inp)
    err = np.abs(out - exp)
    print("max abs err:", err.max(), " rel:", err.max() / np.abs(exp).max())
